# revision 1
# baseline (speedup 1.0000x reference)
"""GNN message passing (gather + scatter-add) on 8 trn2 NeuronCores, v3.

Strategy (dst-sharded, gather via InstDMAGatherAnt on 4 SWDGE queues):
  * Each core owns 12500 dst nodes = 98 tiles of 128. Tiles are processed
    in groups of TPG=14 (7 groups), double-buffered through PSUM banks.
  * x is uploaded once as bf16 [N,128] (256B rows). Edges are binned by
    (group, src-chunk, tile); chunks are <=32768 src rows so int16
    dma_gather indices reach every row. Each (chunk, tile) bin is ONE
    dma_gather instruction (<=1024 indices fits the SWDGE descriptor
    ring); bins rotate across the 4 SWDGE queues so the random-row HBM
    latency is hidden 4-deep. Trailing -1 indices (padding) are trimmed
    by the ucode and cost no descriptors/bytes.
  * One DVE is_equal per (group, chunk) builds the one-hot sel planes;
    one matmul per 128-edge block accumulates psum[dst,f] += sel^T @ msg.
    PSUM zero regions are whole 2KB banks: one start/stop per bank per
    group phase (4 packed dst tiles share a bank's accumulation group).
  * psum -> SBUF via scalar engine (whole-bank copies), HWDGE DMA out.
No collective needed; each core owns its output rows.

Measured on HW: 836968 ns, rel err 1.9e-3 (baseline 2813168 ns). The
span is drain-wall-bound: ~200k gather descriptors/core at ~2.6 ns each
(16 SDMA engines x 4 rings, one outstanding ~165 ns random 256B HBM read
per engine-ring) ~= 520 us, + 117 us count reg_loads + ~160 us fixed/
ring-wait + 25 us ramp + 21 us tail. PE (~170 us) and DVE (~290 us) hide
under the gather. Dead ends verified ON HW this session (CoreSim passes
but HW aborts): >1024 descriptors per gather (runtime-fixed SWDGE ring),
batched multi-register reg_load, and passing the compile-time capacity
as num_idxs_reg (the decode consumes the register). >4 SWDGE queues is a
ucode constant. Remaining ideas, unverified: merge chunk-3's 98 tiny
gathers into per-group gathers with row-0 pads (net ~-10 us); pair-wise
register64 count loads measured SLOWER (983 us); SBUF-source dma_gather
is transpose-only and incompatible with the matmul contraction layout.
"""

import os
import sys

import numpy as np
import ml_dtypes

for _p in ("/opt/trn_rl_repo",):
    if _p not in sys.path:
        sys.path.insert(0, _p)

from concourse import bass, mybir, tile, bacc, library_config  # noqa: E402
from concourse.bass_utils import run_bass_kernel_spmd  # noqa: E402

P = 128
D = 128
N_NODES = 100000
N_CORES = 8
NODES_PER_CORE = N_NODES // N_CORES  # 12500
CHUNK = 32768  # dma_gather idx is int16: chunk-relative indices
TPG = 14  # tiles per group (98 = 7 * 14)

NBUF_M = 4  # msg buffer depth (one buffer per (group, chunk) step)
NBUF_S = 2  # sel buffers
NQ = 4  # SWDGE queues (latency-hiding for the random gather)


def chunk_layout(n_src):
    """[(base, span), ...] covering n_src rows with spans <= CHUNK."""
    spans = []
    b = 0
    while b < n_src:
        s = min(CHUNK, n_src - b)
        spans.append((b, s))
        b += s
    return spans


def build_program(n_src, spans, caps, n_groups, tpg, num_devices):
    """One SPMD program. spans: [(base, span)] per chunk; caps: blocks per
    (chunk, tile) bin. Output rows: n_groups*tpg*128."""
    nch = len(spans)
    assert len(caps) == nch
    scaps = int(sum(caps))
    gblk = tpg * scaps  # blocks per group
    nblk = n_groups * gblk
    ncol_g = tpg * scaps * 8  # idx cols per group (128/16 per block)
    ncol = n_groups * ncol_g
    maxcap = max(caps)
    assert maxcap * P <= 1024, "gather instruction exceeds SWDGE ring"
    nbins = n_groups * nch * tpg
    cum = np.concatenate([[0], np.cumsum(caps)]).astype(int)

    nc = bacc.Bacc(
        "TRN2",
        target_bir_lowering=False,
        debug=False,
        num_devices=num_devices,
        num_swdge_queues=NQ,
    )
    xbf = nc.dram_tensor(
        "xbf", [n_src, D], mybir.dt.bfloat16, kind="ExternalInput"
    ).ap()
    idxT = nc.dram_tensor("idxT", [P, ncol], mybir.dt.int16, kind="ExternalInput").ap()
    dstT = nc.dram_tensor(
        "dstT", [P, nblk], mybir.dt.bfloat16, kind="ExternalInput"
    ).ap()
    iota = nc.dram_tensor(
        "iota", [P, P], mybir.dt.bfloat16, kind="ExternalInput"
    ).ap()
    cntT = nc.dram_tensor("cntT", [P, nbins], mybir.dt.int32, kind="ExternalInput").ap()
    out = nc.dram_tensor(
        "out", [n_groups * tpg * P, D], mybir.dt.float32, kind="ExternalOutput"
    ).ap()

    live = [c for c in range(nch) if caps[c] > 0]
    c_first, c_last = live[0], live[-1]

    with tile.TileContext(nc) as tc:
        with tc.tile_pool(name="sb", bufs=1) as pool, tc.tile_pool(
            name="ps", bufs=1, space="PSUM"
        ) as psp:
            idxs = pool.tile([P, ncol], mybir.dt.int16)
            dsts = pool.tile([P, nblk], mybir.dt.bfloat16)
            iot = pool.tile([P, P], mybir.dt.bfloat16)
            cnts = pool.tile([P, nbins], mybir.dt.int32)
            nc.sync.dma_start(out=idxs[:], in_=idxT[:])
            nc.sync.dma_start(out=dsts[:], in_=dstT[:])
            nc.sync.dma_start(out=iot[:], in_=iota[:])
            nc.sync.dma_start(out=cnts[:], in_=cntT[:])
            nc.gpsimd.load_library(library_config.mlp)

            msg = [
                pool.tile([P, tpg * maxcap, D], mybir.dt.bfloat16, name=f"msg{i}")
                for i in range(NBUF_M)
            ]
            sel = [
                pool.tile([P, tpg * maxcap, P], mybir.dt.bfloat16, name=f"sel{i}")
                for i in range(NBUF_S)
            ]
            stg = [pool.tile([P, tpg * D], mybir.dt.float32, name=f"stg{i}") for i in range(2)]
            # pad slots are never gathered (trailing -1 trim): zero the msg
            # buffers once so stale pad lanes stay finite (sel column is 0).
            for m in msg:
                nc.vector.memset(m[:], 0)
            # PSUM is bank-granular (2KB/partition per bank, 8 banks): pack 4
            # [P, D] f32 accumulation regions per bank.
            bpg = -(-tpg // 4)  # banks per group
            assert 2 * bpg <= 8
            banks = [
                psp.tile([P, 4 * D], dtype=mybir.dt.float32, space="PSUM", name=f"psb{j}")
                for j in range(2 * bpg)
            ]

            def pregion(g, t):
                bk = banks[(g % 2) * bpg + t // 4]
                return bk[:, (t % 4) * D : (t % 4 + 1) * D]

            # per-step register set for the runtime gather counts, filled by
            # ONE batched reg_load per (group, chunk) step (the Pool engine is
            # serial: each gather consumes its register at issue, and the next
            # step's batched load happens after all this step's gathers)
            cregs = [
                [nc.gpsimd.alloc_register(f"cnt{j}_{i}") for i in range(tpg)]
                for j in range(2)
            ]

            step = 0
            gq = 0  # gather queue rotation
            for g in range(n_groups):
                for c in live:
                    nb = tpg * caps[c]
                    km = step % NBUF_M
                    ks = step % NBUF_S
                    mg, sl = msg[km], sel[ks]
                    base, span = spans[c]
                    # one gather per (chunk, tile) bin: trailing -1 idx are
                    # trimmed by the ucode (pads cost nothing); 4-queue
                    # rotation keeps 4 rings of random reads in flight.
                    bin0 = (g * nch + c) * tpg
                    regs = cregs[step % 2]
                    for ti in range(tpg):
                        nc.gpsimd.reg_load(
                            regs[ti], cnts[0:1, bin0 + ti : bin0 + ti + 1]
                        )
                    for ti in range(tpg):
                        n_idx = caps[c] * P
                        coloff = g * ncol_g + (tpg * cum[c] + ti * caps[c]) * 8
                        nc.gpsimd.dma_gather(
                            mg[:, ti * caps[c] : (ti + 1) * caps[c], :],
                            xbf[base : base + span, :],
                            idxs[:, coloff : coloff + n_idx // 16],
                            n_idx,
                            regs[ti],
                            D,
                            queue_num=gq % NQ,
                        )
                        gq += 1
                    # one-hot sel planes for every block of this step
                    blk0 = g * gblk + tpg * cum[c]
                    nc.vector.tensor_tensor(
                        out=sl[:, :nb, :],
                        in0=dsts[:, blk0 : blk0 + nb][:, :, None].to_broadcast(
                            [P, nb, P]
                        ),
                        in1=iot[:, None, :].to_broadcast([P, nb, P]),
                        op=mybir.AluOpType.is_equal,
                    )
                    # b-outer + bank-interleaved tile order: consecutive
                    # matmuls never accumulate into the same psum region (or
                    # bank), so fill/drain pipelines instead of serializing.
                    torder = [t for r in range(4) for t in range(r, tpg, 4)]
                    last_of_bank = {t // 4: t for t in torder}
                    started = set()
                    for b in range(caps[c]):
                        for t in torder:
                            j = t * caps[c] + b
                            # PSUM zero regions are whole 2KB banks: exactly
                            # one start (zeroes the bank) / stop per bank per
                            # group-phase, shared by its 4 packed tiles. PE is
                            # strict FIFO so the bank-zeroing start executes
                            # before its bank-mates accumulate.
                            bank = t // 4
                            start = (
                                c == c_first and b == 0 and bank not in started
                            )
                            if start:
                                started.add(bank)
                            nc.tensor.matmul(
                                out=pregion(g, t),
                                lhsT=sl[:, j, :],
                                rhs=mg[:, j, :],
                                start=start,
                                stop=(
                                    c == c_last
                                    and b == caps[c] - 1
                                    and t == last_of_bank[bank]
                                ),
                            )
                    step += 1
                sg = stg[g % 2]
                # whole-bank psum->SBUF copies: the full-bank read AP also
                # orders the next phase's bank-zeroing start matmul (which
                # clears all 2KB) after every tile's columns are drained.
                for k in range(bpg):
                    w = min(4, tpg - 4 * k)
                    bk = banks[(g % 2) * bpg + k]
                    nc.scalar.copy(
                        sg[:, 4 * k * D : (4 * k + w) * D], bk[:, : w * D]
                    )
                for t in range(tpg):
                    r0 = (g * tpg + t) * P
                    nc.sync.dma_start(
                        out=out[r0 : r0 + P, :], in_=sg[:, t * D : (t + 1) * D]
                    )

    # Tile assigns DMASW sem lanes in its own (scheduled) order, and a lane
    # must stay on one SWDGE queue. Re-derive each gather's queue from its
    # assigned lane so sem<->queue stays consistent.
    for blk in nc.main_func.blocks:
        for ins in blk.instructions:
            if isinstance(ins, mybir.InstDMAGatherAnt):
                si = ins.sync_info
                if si and si.on_update:
                    name = si.on_update[0].ant_name  # e.g. DMASW7_49
                    lane = int(name.split("_")[0][len("DMASW") :])
                    ins.queue_num = lane % NQ
    nc.compile()
    return nc


def prep_core(src, dst, core_base, n_dst, spans, caps, n_groups, tpg):
    """Bin one core's edges (src global, dst core-relative via core_base) into
    the (group, chunk, tile) slot layout. Returns (idxT, dstT, cntT)."""
    nch = len(spans)
    scaps = int(sum(caps))
    gblk = tpg * scaps
    nblk = n_groups * gblk
    cum = np.concatenate([[0], np.cumsum(caps)]).astype(np.int64)

    m = (dst >= core_base) & (dst < core_base + n_dst)
    es = src[m].astype(np.int64)
    rel = (dst[m] - core_base).astype(np.int64)
    t = rel >> 7
    i = t % tpg
    g = t // tpg
    c = np.minimum(es // CHUNK, nch - 1)
    bases = np.array([b for b, s in spans], dtype=np.int64)
    sr = es - bases[c]

    binkey = (g * nch + c) * tpg + i
    # secondary sort by src: ascending addresses within each gather's
    # descriptor stream (DRAM row-buffer locality, adjacent duplicates)
    order = np.lexsort((sr, binkey))
    sr, rel, binkey = sr[order], rel[order], binkey[order]
    nbins = n_groups * nch * tpg
    counts = np.bincount(binkey, minlength=nbins)
    caps_arr = np.asarray(caps, dtype=np.int64)
    kk = np.arange(nbins)
    ck = (kk // tpg) % nch
    capacity = caps_arr[ck] * P
    if (counts > capacity).any():
        raise ValueError("caps too small")
    starts = np.zeros(nbins, dtype=np.int64)
    starts[1:] = np.cumsum(counts)[:-1]
    pos = np.arange(len(sr)) - starts[binkey]
    gk = kk // (nch * tpg)
    ik = kk % tpg
    bin_base = (gk * gblk + tpg * cum[ck] + ik * caps_arr[ck]) * P
    slot = bin_base[binkey] + pos

    total = nblk * P
    srcflat = np.full(total, -1, np.int64)  # -1 = pad (trimmed, not gathered)
    dstflat = np.full(total, -1, np.int64)
    srcflat[slot] = sr
    dstflat[slot] = rel & 127
    cnt = counts.astype(np.int64)
    # empty bins would break the trailing-trim (no valid last idx): give
    # them one real (but sel-zeroed) gather of row 0.
    empty = np.flatnonzero(cnt == 0)
    if len(empty):
        srcflat[bin_base[empty]] = 0
        cnt[empty] = 1

    # idx planes: per-bin segment, wrapped 16 partitions
    ncol_g = tpg * scaps * 8
    ncol = n_groups * ncol_g
    idxT = np.empty((16, ncol), np.int16)
    seg = srcflat.reshape(-1, 16)  # every bin is a multiple of 128 slots
    idxT[:, :] = seg.T.reshape(16, ncol)
    idxT = np.tile(idxT, (8, 1))
    dstT = np.ascontiguousarray(
        dstflat.reshape(nblk, P).T.astype(ml_dtypes.bfloat16)
    )
    cntT = np.tile(cnt.astype(np.int32), (P, 1))
    return idxT, dstT, cntT


def compute_caps(src, dst, n_cores, n_dst_per_core, spans, n_tiles):
    """Max blocks over all (core, chunk, tile) bins, per chunk."""
    src = np.asarray(src, dtype=np.int64)
    dst = np.asarray(dst, dtype=np.int64)
    nch = len(spans)
    core = dst // n_dst_per_core
    t = (dst - core * n_dst_per_core) >> 7
    c = np.minimum(src // CHUNK, nch - 1)
    binid = (core * nch + c) * n_tiles + t
    counts = np.bincount(binid, minlength=n_cores * nch * n_tiles)
    counts = counts.reshape(n_cores, nch, n_tiles)
    per_chunk = counts.max(axis=(0, 2))
    return [int(-(-int(v) // P)) for v in per_chunk]


_cache = {}


def kernel(x, edge_index):
    x = np.asarray(x, dtype=np.float32)
    edge_index = np.asarray(edge_index)
    src = edge_index[0].astype(np.int64)
    dst = edge_index[1].astype(np.int64)

    n_tiles = -(-NODES_PER_CORE // P)  # 98
    n_groups = n_tiles // TPG  # 7
    spans = chunk_layout(N_NODES)
    caps = compute_caps(src, dst, N_CORES, NODES_PER_CORE, spans, n_tiles)
    caps = [min(c, 1024 // P) for c in caps]  # ring limit guard (raises later)

    key = (N_NODES, tuple(caps), n_groups, TPG)
    if key not in _cache:
        _cache[key] = build_program(N_NODES, spans, caps, n_groups, TPG, N_CORES)
    nc = _cache[key]

    xbf = np.ascontiguousarray(x.astype(ml_dtypes.bfloat16))
    iota = np.tile(
        np.arange(P, dtype=np.float32).astype(ml_dtypes.bfloat16), (P, 1)
    )
    in_maps = []
    for k in range(N_CORES):
        idxT, dstT, cntT = prep_core(
            src, dst, k * NODES_PER_CORE, NODES_PER_CORE, spans, caps, n_groups, TPG
        )
        in_maps.append(
            {"xbf": xbf, "idxT": idxT, "dstT": dstT, "iota": iota, "cntT": cntT}
        )

    trace = bool(int(os.environ.get("KERNEL_TRACE", "0")))
    res = run_bass_kernel_spmd(
        nc, in_maps, core_ids=list(range(N_CORES)), trace=trace
    )
    if trace:
        kernel.last_results = res
    outs = [res.results[c]["out"][:NODES_PER_CORE] for c in range(N_CORES)]
    return np.ascontiguousarray(np.concatenate(outs, axis=0))



# revision 4
# speedup vs baseline: 1.1441x; 1.1441x over previous
"""GNN message passing (gather + scatter-add) on 8 trn2 NeuronCores, v4.

Strategy (dst-sharded, gather via InstDMAGatherAnt on 4 SWDGE queues):
  * The host ASSIGNS dst nodes to (core, tile, partition) slots with a
    load balancer (serpentine deal on degree + per-(tile,chunk) repair
    swaps) so every (group, chunk, tile) bin has ~equal edge count.
    This shrinks the SPMD-uniform bin capacities to ~mean (the baseline
    paid max-over-784-Poisson-bins), cutting msg/sel SBUF and blocks.
  * 112 tiles of 128 dst slots per core (TPG=16, 7 groups; 4 PSUM banks
    per group, 2 groups in flight = 8 banks). x stays in HBM as bf16;
    edges are binned by (group, chunk=25000 src rows, tile); each bin is
    gathered by ONE dma_gather (chunk-relative int16 idx), bins for
    adjacent tile pairs are merged into one gather (<=1024 ring descs)
    with mid-stream idx-0 pads (dst code 255 => sel column 0) and
    trailing -1 (trimmed by ucode via the per-core count register).
  * Descriptors can be 512B (DESC_ROWS=2: each desc fetches rows
    [src, src+2), the matmul reads cols 0:128) - measured ~16% faster
    per descriptor than 256B on the SWDGE queue pipeline.
  * One DVE is_equal per (group, chunk) step builds one-hot sel planes;
    one matmul per 128-slot block accumulates psum[dst,f] += sel^T@msg.
    Stale slots (count < capacity) keep old finite bf16 data and get
    sel 0. PSUM start/stop once per bank per group phase.
  * psum -> SBUF via scalar engine (whole-bank copies), HWDGE DMA out;
    host un-permutes rows via the balancer's node map.
No collective needed; each core owns its output rows.
"""

import os
import sys

import numpy as np
import ml_dtypes

for _p in ("/opt/trn_rl_repo",):
    if _p not in sys.path:
        sys.path.insert(0, _p)

import bass_rust  # noqa: E402
from concourse import bass, mybir, tile, bacc, library_config  # noqa: E402
from concourse.bass_utils import run_bass_kernel_spmd  # noqa: E402

P = 128
D = 128
N_NODES = 100000
N_CORES = 8

NBUF_S = 2  # sel buffers


def make_chunks(n_src, chunk):
    spans = []
    b = 0
    while b < n_src:
        s = min(chunk, n_src - b)
        spans.append((b, s))
        b += s
    return spans


def balance_nodes(src, dst, n_cores, tiles, spans):
    """Assign each dst node a (core, tile, partition) slot, balancing the
    per-(tile, chunk) edge counts. Returns node_slot [N] int64 encoding
    core*tiles*128 + tile*128 + p, with every tile holding <=128 nodes."""
    n = N_NODES
    nch = len(spans)
    ntile = n_cores * tiles
    # per-node per-chunk degree
    cid = np.minimum(src // spans[0][1], nch - 1)
    degc = np.zeros((nch, n), np.int32)
    for c in range(nch):
        degc[c] = np.bincount(dst[cid == c], minlength=n)
    deg = degc.sum(axis=0)

    # serpentine deal on total degree: round r gives one node to each tile,
    # pairing heaviest remaining nodes with lightest tiles.
    order = np.argsort(-deg, kind="stable")
    loads = np.zeros(ntile, np.int64)
    fill = np.zeros(ntile, np.int32)
    assign = np.empty(n, np.int64)
    pos = 0
    while pos < n:
        batch = order[pos : pos + ntile]
        tl = np.argsort(loads, kind="stable")[: len(batch)]
        assign[batch] = tl
        loads[tl] += deg[batch]
        fill[tl] += 1
        pos += len(batch)

    # repair pass: per-(tile, chunk) loads; swap high-deg nodes out of
    # overloaded bins into the lightest tiles (matched by total degree).
    cl = np.zeros((ntile, nch), np.int64)
    for c in range(nch):
        np.add.at(cl[:, c], assign, degc[c])
    for _ in range(200):
        worst = np.unravel_index(np.argmax(cl), cl.shape)
        t0, c0 = int(worst[0]), int(worst[1])
        lim = cl.max(axis=1).mean() + 8
        if cl[t0, c0] <= lim:
            break
        cand = np.flatnonzero(assign == t0)
        mover = cand[np.argmax(degc[c0, cand])]
        t1 = int(np.argmin(cl[:, c0] + (fill >= 128) * (1 << 40)))
        # swap mover with a node in t1 of similar total degree but low c0 deg
        cand1 = np.flatnonzero(assign == t1)
        recv = cand1[np.argmin(degc[c0, cand1].astype(np.int64) * (1 << 20) - deg[cand1])]
        assign[mover], assign[recv] = t1, t0
        cl[t0] += degc[:, recv] - degc[:, mover]
        cl[t1] += degc[:, mover] - degc[:, recv]

    # partition index within tile
    order2 = np.argsort(assign, kind="stable")
    idx_in_tile = np.empty(n, np.int64)
    start = 0
    counts = np.bincount(assign, minlength=ntile)
    assert counts.max() <= 128
    off = np.concatenate([[0], np.cumsum(counts)])
    ranks = np.arange(n) - off[assign[order2]]
    idx_in_tile[order2] = ranks
    node_slot = assign * P + idx_in_tile
    return node_slot  # global slot id: (core*tiles + tile)*128 + p


def build_program(spans, caps, n_groups, tpg, num_devices, desc_rows, nbuf_m, nq):
    """caps: int array [n_groups, nch, tpg] = blocks per bin (uniform across
    cores). Gathers merge adjacent tile pairs. Output rows: n_groups*tpg*128."""
    nch = len(spans)
    E = D * desc_rows  # gathered elems per slot
    blocks = np.asarray(caps)  # [g][c][t]
    nblk = int(blocks.sum())
    step_blocks = blocks.sum(axis=2)  # [g][c]
    max_nb = int(step_blocks.max())
    nbins = n_groups * nch * tpg
    # merged gathers: pairs of adjacent tiles
    npair = (tpg + 1) // 2
    ngath = n_groups * nch * npair

    nc = bacc.Bacc(
        "TRN2",
        target_bir_lowering=False,
        debug=False,
        num_devices=num_devices,
        num_swdge_queues=nq,
    )
    n_src = spans[-1][0] + spans[-1][1]
    xbf = nc.dram_tensor(
        "xbf", [n_src + 2 * desc_rows, D], mybir.dt.bfloat16, kind="ExternalInput"
    ).ap()
    ncol = nblk * P // 16
    idxT = nc.dram_tensor("idxT", [P, ncol], mybir.dt.int16, kind="ExternalInput").ap()
    dstT = nc.dram_tensor(
        "dstT", [P, nblk], mybir.dt.bfloat16, kind="ExternalInput"
    ).ap()
    iota = nc.dram_tensor(
        "iota", [P, P], mybir.dt.bfloat16, kind="ExternalInput"
    ).ap()
    cntT = nc.dram_tensor("cntT", [P, ngath], mybir.dt.int32, kind="ExternalInput").ap()
    out = nc.dram_tensor(
        "out", [n_groups * tpg * P, D], mybir.dt.float32, kind="ExternalOutput"
    ).ap()

    # slot offset (in blocks) of each bin, ordered (g, c, t)
    boff = np.zeros(nbins + 1, np.int64)
    boff[1:] = np.cumsum(blocks.reshape(-1))

    def bin_id(g, c, t):
        return (g * nch + c) * tpg + t

    with tile.TileContext(nc) as tc:
        with tc.tile_pool(name="sb", bufs=1) as pool, tc.tile_pool(
            name="ps", bufs=1, space="PSUM"
        ) as psp:
            idxs = pool.tile([P, ncol], mybir.dt.int16)
            dsts = pool.tile([P, nblk], mybir.dt.bfloat16)
            iot = pool.tile([P, P], mybir.dt.bfloat16)
            cnts = pool.tile([P, ngath], mybir.dt.int32)
            nc.sync.dma_start(out=idxs[:], in_=idxT[:])
            nc.sync.dma_start(out=dsts[:], in_=dstT[:])
            nc.sync.dma_start(out=iot[:], in_=iota[:])
            nc.sync.dma_start(out=cnts[:], in_=cntT[:])
            nc.gpsimd.load_library(library_config.mlp)

            msg = [
                pool.tile([P, max_nb, E], mybir.dt.bfloat16, name=f"msg{i}")
                for i in range(nbuf_m)
            ]
            sel = [
                pool.tile([P, max_nb, P], mybir.dt.bfloat16, name=f"sel{i}")
                for i in range(NBUF_S)
            ]
            stg = [
                pool.tile([P, tpg * D], mybir.dt.float32, name=f"stg{i}")
                for i in range(2)
            ]
            # stale slots must stay finite: zero the msg buffers once.
            for m in msg:
                nc.vector.memset(m[:], 0)
            bpg = -(-tpg // 4)  # banks per group
            assert 2 * bpg <= 8
            banks = [
                psp.tile([P, 4 * D], dtype=mybir.dt.float32, space="PSUM", name=f"psb{j}")
                for j in range(2 * bpg)
            ]

            def pregion(g, t):
                bk = banks[(g % 2) * bpg + t // 4]
                return bk[:, (t % 4) * D : (t % 4 + 1) * D]

            cregs = [
                [nc.gpsimd.alloc_register(f"cnt{j}_{i}") for i in range(npair)]
                for j in range(2)
            ]

            step = 0
            gq = 0
            for g in range(n_groups):
                for c in range(nch):
                    km = step % nbuf_m
                    ks = step % NBUF_S
                    mg, sl = msg[km], sel[ks]
                    base, span = spans[c]
                    nb = int(step_blocks[g, c])
                    sb0 = boff[bin_id(g, c, 0)]  # first block of this step
                    regs = cregs[step % 2]
                    for pi in range(npair):
                        gi = (g * nch + c) * npair + pi
                        nc.gpsimd.reg_load(
                            regs[pi], cnts[0:1, gi : gi + 1]
                        )
                    inap = xbf[base : base + span + 2 * desc_rows, :]
                    if desc_rows > 1:
                        # overlapping window view: row i -> elems [i*D, i*D+E)
                        inap = inap.copy()
                        inap.ap = bass_rust.VecI64Pair(
                            [(D, span + desc_rows), (1, E)]
                        )
                    for pi in range(npair):
                        t0 = 2 * pi
                        t1 = min(2 * pi + 1, tpg - 1)
                        b0 = boff[bin_id(g, c, t0)]
                        bend = boff[bin_id(g, c, t1)] + blocks[g, c, t1]
                        nslot = int(bend - b0) * P
                        coloff = int(b0) * P // 16
                        nc.gpsimd.dma_gather(
                            mg[:, int(b0 - sb0) : int(bend - sb0), :],
                            inap,
                            idxs[:, coloff : coloff + nslot // 16],
                            nslot,
                            regs[pi],
                            E,
                            elem_step=D,
                            queue_num=gq % nq,
                        )
                        gq += 1
                    nc.vector.tensor_tensor(
                        out=sl[:, :nb, :],
                        in0=dsts[:, int(sb0) : int(sb0 + nb)][:, :, None].to_broadcast(
                            [P, nb, P]
                        ),
                        in1=iot[:, None, :].to_broadcast([P, nb, P]),
                        op=mybir.AluOpType.is_equal,
                    )
                    # bank-interleaved tile order, block-outer: consecutive
                    # matmuls never hit the same psum region/bank.
                    torder = [t for r in range(4) for t in range(r, tpg, 4)]
                    last_of_bank = {}
                    maxb = int(blocks[g, c].max())
                    for b in range(maxb):
                        for t in torder:
                            if b < blocks[g, c, t]:
                                last_of_bank[t // 4] = (t, b)
                    started = set()
                    for b in range(maxb):
                        for t in torder:
                            if b >= blocks[g, c, t]:
                                continue
                            j = int(boff[bin_id(g, c, t)] - sb0) + b
                            bank = t // 4
                            start = c == 0 and b == 0 and bank not in started
                            if start:
                                started.add(bank)
                            nc.tensor.matmul(
                                out=pregion(g, t),
                                lhsT=sl[:, j, :],
                                rhs=mg[:, j, 0:D],
                                start=start,
                                stop=(
                                    c == nch - 1
                                    and last_of_bank[bank] == (t, b)
                                ),
                            )
                    step += 1
                sg = stg[g % 2]
                for k in range(bpg):
                    w = min(4, tpg - 4 * k)
                    bk = banks[(g % 2) * bpg + k]
                    nc.scalar.copy(
                        sg[:, 4 * k * D : (4 * k + w) * D], bk[:, : w * D]
                    )
                for t in range(tpg):
                    r0 = (g * tpg + t) * P
                    nc.sync.dma_start(
                        out=out[r0 : r0 + P, :], in_=sg[:, t * D : (t + 1) * D]
                    )

    for blk in nc.main_func.blocks:
        for ins in blk.instructions:
            if isinstance(ins, mybir.InstDMAGatherAnt):
                si = ins.sync_info
                if si and si.on_update:
                    name = si.on_update[0].ant_name
                    lane = int(name.split("_")[0][len("DMASW") :])
                    ins.queue_num = lane % nq
    nc.compile()
    return nc


def prep_core(src, rel, spans, caps, n_groups, tpg):
    """Bin one core's edges (src global, rel = tile*128+p core-relative slot)
    into the (group, chunk, tile) layout. Returns (idxT, dstT, cntT)."""
    nch = len(spans)
    blocks = np.asarray(caps)
    nblk = int(blocks.sum())
    nbins = n_groups * nch * tpg
    npair = (tpg + 1) // 2
    boff = np.zeros(nbins + 1, np.int64)
    boff[1:] = np.cumsum(blocks.reshape(-1))

    t = rel >> 7
    g = t // tpg
    ti = t % tpg
    chunk = spans[0][1]
    c = np.minimum(src // chunk, nch - 1)
    bases = np.array([b for b, s in spans], dtype=np.int64)
    sr = src - bases[c]

    binkey = (g * nch + c) * tpg + ti
    order = np.lexsort((sr, binkey))
    sr, relo, binkey = sr[order], rel[order], binkey[order]
    counts = np.bincount(binkey, minlength=nbins)
    capacity = blocks.reshape(-1) * P
    if (counts > capacity).any():
        raise ValueError("caps too small")
    starts = np.zeros(nbins, np.int64)
    starts[1:] = np.cumsum(counts)[:-1]
    pos = np.arange(len(sr)) - starts[binkey]
    slot = boff[binkey] * P + pos

    total = nblk * P
    srcflat = np.full(total, -1, np.int64)
    dstflat = np.full(total, 255, np.int64)  # 255 = stale (sel col 0)
    srcflat[slot] = sr
    dstflat[slot] = relo & 127

    # merged pair gathers: mid-bins pad with idx 0 (valid, sel-zeroed);
    # the final bin of each pair keeps -1 (trailing trim via count reg).
    cnt = np.zeros(n_groups * nch * npair, np.int64)
    for gg in range(n_groups):
        for cc in range(nch):
            for pi in range(npair):
                t0 = 2 * pi
                t1 = min(2 * pi + 1, tpg - 1)
                k0 = (gg * nch + cc) * tpg + t0
                k1 = (gg * nch + cc) * tpg + t1
                gi = (gg * nch + cc) * npair + pi
                if t1 > t0:
                    # pad bin t0's tail (idx 0) up to its capacity
                    s0, e0 = boff[k0] * P, (boff[k0] + blocks[gg, cc, t0]) * P
                    tail = srcflat[s0:e0]
                    tail[tail < 0] = 0
                    cnt[gi] = (e0 - s0) + counts[k1]
                    if counts[k1] == 0:
                        # need a valid final idx for the trim loop
                        srcflat[boff[k1] * P] = 0
                        cnt[gi] = (e0 - s0) + 1
                else:
                    cnt[gi] = counts[k0]
                    if counts[k0] == 0:
                        srcflat[boff[k0] * P] = 0
                        cnt[gi] = 1

    idxT = np.empty((16, total // 16), np.int16)
    seg = srcflat.reshape(-1, 16)
    idxT[:, :] = seg.T.reshape(16, total // 16)
    idxT = np.tile(idxT, (8, 1))
    dstT = np.ascontiguousarray(
        dstflat.reshape(nblk, P).T.astype(ml_dtypes.bfloat16)
    )
    cntT = np.tile(cnt.astype(np.int32), (P, 1))
    return idxT, dstT, cntT


def compute_caps(binned_counts):
    """binned_counts: [n_cores, n_groups, nch, tpg] -> blocks per bin
    (max over cores, ceil /128)."""
    mx = binned_counts.max(axis=0)
    return np.maximum(1, -(-mx // P)).astype(np.int64)


_cache = {}


def kernel(x, edge_index):
    TILES = 112
    TPG = 16
    CHUNK = 25000
    DESC_ROWS = int(os.environ.get("KERNEL_DESC_ROWS", "1"))
    NBUF_M = int(os.environ.get("KERNEL_NBUF_M", "3"))
    NQ = 4

    x = np.asarray(x, dtype=np.float32)
    edge_index = np.asarray(edge_index)
    src = edge_index[0].astype(np.int64)
    dst = edge_index[1].astype(np.int64)

    n_groups = TILES // TPG
    spans = make_chunks(N_NODES, CHUNK)
    nch = len(spans)

    node_slot = balance_nodes(src, dst, N_CORES, TILES, spans)
    eslot = node_slot[dst]
    ecore = eslot // (TILES * P)
    erel = eslot % (TILES * P)

    # per-core bin counts for caps
    cid = np.minimum(src // CHUNK, nch - 1)
    t = erel >> 7
    bk = ((ecore * n_groups + t // TPG) * nch + cid) * TPG + (t % TPG)
    bc = np.bincount(bk, minlength=N_CORES * n_groups * nch * TPG).reshape(
        N_CORES, n_groups, nch, TPG
    )
    caps = compute_caps(bc)

    key = (caps.tobytes(), n_groups, TPG, DESC_ROWS, NBUF_M)
    if key not in _cache:
        _cache[key] = build_program(
            spans, caps, n_groups, TPG, N_CORES, DESC_ROWS, NBUF_M, NQ
        )
    nc = _cache[key]

    xbf = np.zeros((N_NODES + 2 * DESC_ROWS, D), ml_dtypes.bfloat16)
    xbf[:N_NODES] = x.astype(ml_dtypes.bfloat16)
    iota = np.tile(
        np.arange(P, dtype=np.float32).astype(ml_dtypes.bfloat16), (P, 1)
    )
    in_maps = []
    for k in range(N_CORES):
        m = ecore == k
        idxT, dstT, cntT = prep_core(
            src[m], erel[m], spans, caps, n_groups, TPG
        )
        in_maps.append(
            {"xbf": xbf, "idxT": idxT, "dstT": dstT, "iota": iota, "cntT": cntT}
        )

    trace = bool(int(os.environ.get("KERNEL_TRACE", "0")))
    res = run_bass_kernel_spmd(
        nc, in_maps, core_ids=list(range(N_CORES)), trace=trace
    )
    if trace:
        kernel.last_results = res
    dev = np.stack([res.results[c]["out"] for c in range(N_CORES)])  # [8, T*128, D]
    full = np.empty((N_NODES, D), np.float32)
    full[:] = dev.reshape(N_CORES * TILES * P, D)[node_slot]
    return np.ascontiguousarray(full)


# revision 5
# speedup vs baseline: 1.2183x; 1.0649x over previous
"""GNN message passing (gather + scatter-add) on 8 trn2 NeuronCores, v4.

Strategy (dst-sharded, gather via InstDMAGatherAnt on 4 SWDGE queues):
  * The host ASSIGNS dst nodes to (core, tile, partition) slots with a
    load balancer (serpentine deal on degree + per-(tile,chunk) repair
    swaps) so every (group, chunk, tile) bin has ~equal edge count.
    This shrinks the SPMD-uniform bin capacities to ~mean (the baseline
    paid max-over-784-Poisson-bins), cutting msg/sel SBUF and blocks.
  * 112 tiles of 128 dst slots per core (TPG=16, 7 groups; 4 PSUM banks
    per group, 2 groups in flight = 8 banks). x stays in HBM as bf16;
    edges are binned by (group, chunk=25000 src rows, tile); each bin is
    gathered by ONE dma_gather (chunk-relative int16 idx), bins for
    adjacent tile pairs are merged into one gather (<=1024 ring descs)
    with mid-stream idx-0 pads (dst code 255 => sel column 0) and
    trailing -1 (trimmed by ucode via the per-core count register).
  * Descriptors can be 512B (DESC_ROWS=2: each desc fetches rows
    [src, src+2), the matmul reads cols 0:128) - measured ~16% faster
    per descriptor than 256B on the SWDGE queue pipeline.
  * One DVE is_equal per (group, chunk) step builds one-hot sel planes;
    one matmul per 128-slot block accumulates psum[dst,f] += sel^T@msg.
    Stale slots (count < capacity) keep old finite bf16 data and get
    sel 0. PSUM start/stop once per bank per group phase.
  * psum -> SBUF via scalar engine (whole-bank copies), HWDGE DMA out;
    host un-permutes rows via the balancer's node map.
No collective needed; each core owns its output rows.
"""

import os
import sys

import numpy as np
import ml_dtypes

for _p in ("/opt/trn_rl_repo",):
    if _p not in sys.path:
        sys.path.insert(0, _p)

import bass_rust  # noqa: E402
from concourse import bass, mybir, tile, bacc, library_config  # noqa: E402
from concourse.bass_utils import run_bass_kernel_spmd  # noqa: E402

P = 128
D = 128
N_NODES = 100000
N_CORES = 8

NBUF_S = 2  # sel buffers


def make_chunks(n_src, chunk):
    spans = []
    b = 0
    while b < n_src:
        s = min(chunk, n_src - b)
        spans.append((b, s))
        b += s
    return spans


def balance_nodes(src, dst, n_cores, tiles, spans):
    """Assign each dst node a (core, tile, partition) slot, balancing the
    per-(tile, chunk) edge counts. Returns node_slot [N] int64 encoding
    core*tiles*128 + tile*128 + p, with every tile holding <=128 nodes."""
    n = N_NODES
    nch = len(spans)
    ntile = n_cores * tiles
    # per-node per-chunk degree
    cid = np.minimum(src // spans[0][1], nch - 1)
    degc = np.zeros((nch, n), np.int32)
    for c in range(nch):
        degc[c] = np.bincount(dst[cid == c], minlength=n)
    deg = degc.sum(axis=0)

    # serpentine deal on total degree: round r gives one node to each tile,
    # pairing heaviest remaining nodes with lightest tiles.
    order = np.argsort(-deg, kind="stable")
    loads = np.zeros(ntile, np.int64)
    fill = np.zeros(ntile, np.int32)
    assign = np.empty(n, np.int64)
    pos = 0
    while pos < n:
        batch = order[pos : pos + ntile]
        tl = np.argsort(loads, kind="stable")[: len(batch)]
        assign[batch] = tl
        loads[tl] += deg[batch]
        fill[tl] += 1
        pos += len(batch)

    # repair pass: per-(tile, chunk) loads; swap high-deg nodes out of
    # overloaded bins into the lightest tiles (matched by total degree).
    cl = np.zeros((ntile, nch), np.int64)
    for c in range(nch):
        np.add.at(cl[:, c], assign, degc[c])
    for _ in range(200):
        worst = np.unravel_index(np.argmax(cl), cl.shape)
        t0, c0 = int(worst[0]), int(worst[1])
        lim = cl.max(axis=1).mean() + 8
        if cl[t0, c0] <= lim:
            break
        cand = np.flatnonzero(assign == t0)
        mover = cand[np.argmax(degc[c0, cand])]
        t1 = int(np.argmin(cl[:, c0] + (fill >= 128) * (1 << 40)))
        # swap mover with a node in t1 of similar total degree but low c0 deg
        cand1 = np.flatnonzero(assign == t1)
        recv = cand1[np.argmin(degc[c0, cand1].astype(np.int64) * (1 << 20) - deg[cand1])]
        assign[mover], assign[recv] = t1, t0
        cl[t0] += degc[:, recv] - degc[:, mover]
        cl[t1] += degc[:, mover] - degc[:, recv]

    # partition index within tile
    order2 = np.argsort(assign, kind="stable")
    idx_in_tile = np.empty(n, np.int64)
    start = 0
    counts = np.bincount(assign, minlength=ntile)
    assert counts.max() <= 128
    off = np.concatenate([[0], np.cumsum(counts)])
    ranks = np.arange(n) - off[assign[order2]]
    idx_in_tile[order2] = ranks
    node_slot = assign * P + idx_in_tile
    return node_slot  # global slot id: (core*tiles + tile)*128 + p


def build_program(spans, caps, n_groups, tpg, num_devices, desc_rows, nbuf_m, nq):
    """caps: int array [n_groups, nch, tpg] = blocks per bin (uniform across
    cores). Gathers merge adjacent tile pairs. Output rows: n_groups*tpg*128."""
    nch = len(spans)
    E = D * desc_rows  # gathered elems per slot
    blocks = np.asarray(caps)  # [g][c][t]
    nblk = int(blocks.sum())
    step_blocks = blocks.sum(axis=2)  # [g][c]
    max_nb = int(step_blocks.max())
    nbins = n_groups * nch * tpg
    # merged gathers: pairs of adjacent tiles
    npair = (tpg + 1) // 2
    ngath = n_groups * nch * npair

    nc = bacc.Bacc(
        "TRN2",
        target_bir_lowering=False,
        debug=False,
        num_devices=num_devices,
        num_swdge_queues=nq,
        # ring of 2048 descriptors per SWDGE queue: two 1024-desc gathers in
        # flight per queue (deeper gen/drain pipelining, measured ~15% faster
        # per descriptor than the default 1024-desc ring).
        dynamic_dma_scratch_size=32768,
    )
    n_src = spans[-1][0] + spans[-1][1]
    xbf = nc.dram_tensor(
        "xbf", [n_src + 2 * desc_rows, D], mybir.dt.bfloat16, kind="ExternalInput"
    ).ap()
    ncol = nblk * P // 16
    idxT = nc.dram_tensor("idxT", [P, ncol], mybir.dt.int16, kind="ExternalInput").ap()
    dstT = nc.dram_tensor(
        "dstT", [P, nblk], mybir.dt.bfloat16, kind="ExternalInput"
    ).ap()
    iota = nc.dram_tensor(
        "iota", [P, P], mybir.dt.bfloat16, kind="ExternalInput"
    ).ap()
    cntT = nc.dram_tensor("cntT", [P, ngath], mybir.dt.int32, kind="ExternalInput").ap()
    out = nc.dram_tensor(
        "out", [n_groups * tpg * P, D], mybir.dt.float32, kind="ExternalOutput"
    ).ap()

    # slot offset (in blocks) of each bin, ordered (g, c, t)
    boff = np.zeros(nbins + 1, np.int64)
    boff[1:] = np.cumsum(blocks.reshape(-1))

    def bin_id(g, c, t):
        return (g * nch + c) * tpg + t

    with tile.TileContext(nc) as tc:
        with tc.tile_pool(name="sb", bufs=1) as pool, tc.tile_pool(
            name="ps", bufs=1, space="PSUM"
        ) as psp:
            idxs = pool.tile([P, ncol], mybir.dt.int16)
            dsts = pool.tile([P, nblk], mybir.dt.bfloat16)
            iot = pool.tile([P, P], mybir.dt.bfloat16)
            cnts = pool.tile([P, ngath], mybir.dt.int32)
            nc.sync.dma_start(out=idxs[:], in_=idxT[:])
            nc.sync.dma_start(out=dsts[:], in_=dstT[:])
            nc.sync.dma_start(out=iot[:], in_=iota[:])
            nc.sync.dma_start(out=cnts[:], in_=cntT[:])
            nc.gpsimd.load_library(library_config.mlp)

            msg = [
                pool.tile([P, max_nb, E], mybir.dt.bfloat16, name=f"msg{i}")
                for i in range(nbuf_m)
            ]
            sel = [
                pool.tile([P, max_nb, P], mybir.dt.bfloat16, name=f"sel{i}")
                for i in range(NBUF_S)
            ]
            stg = [
                pool.tile([P, tpg * D], mybir.dt.float32, name=f"stg{i}")
                for i in range(2)
            ]
            # stale slots must stay finite: zero the msg buffers once.
            for m in msg:
                nc.vector.memset(m[:], 0)
            bpg = -(-tpg // 4)  # banks per group
            assert 2 * bpg <= 8
            banks = [
                psp.tile([P, 4 * D], dtype=mybir.dt.float32, space="PSUM", name=f"psb{j}")
                for j in range(2 * bpg)
            ]

            def pregion(g, t):
                bk = banks[(g % 2) * bpg + t // 4]
                return bk[:, (t % 4) * D : (t % 4 + 1) * D]

            cregs = [
                [nc.gpsimd.alloc_register(f"cnt{j}_{i}") for i in range(npair)]
                for j in range(2)
            ]

            step = 0
            gq = 0
            for g in range(n_groups):
                for c in range(nch):
                    km = step % nbuf_m
                    ks = step % NBUF_S
                    mg, sl = msg[km], sel[ks]
                    base, span = spans[c]
                    nb = int(step_blocks[g, c])
                    sb0 = boff[bin_id(g, c, 0)]  # first block of this step
                    regs = cregs[step % 2]
                    for pi in range(npair):
                        gi = (g * nch + c) * npair + pi
                        nc.gpsimd.reg_load(
                            regs[pi], cnts[0:1, gi : gi + 1]
                        )
                    inap = xbf[base : base + span + 2 * desc_rows, :]
                    if desc_rows > 1:
                        # overlapping window view: row i -> elems [i*D, i*D+E)
                        inap = inap.copy()
                        inap.ap = bass_rust.VecI64Pair(
                            [(D, span + desc_rows), (1, E)]
                        )
                    for pi in range(npair):
                        t0 = 2 * pi
                        t1 = min(2 * pi + 1, tpg - 1)
                        b0 = boff[bin_id(g, c, t0)]
                        bend = boff[bin_id(g, c, t1)] + blocks[g, c, t1]
                        nslot = int(bend - b0) * P
                        coloff = int(b0) * P // 16
                        nc.gpsimd.dma_gather(
                            mg[:, int(b0 - sb0) : int(bend - sb0), :],
                            inap,
                            idxs[:, coloff : coloff + nslot // 16],
                            nslot,
                            regs[pi],
                            E,
                            elem_step=D,
                            queue_num=gq % nq,
                        )
                        gq += 1
                    nc.vector.tensor_tensor(
                        out=sl[:, :nb, :],
                        in0=dsts[:, int(sb0) : int(sb0 + nb)][:, :, None].to_broadcast(
                            [P, nb, P]
                        ),
                        in1=iot[:, None, :].to_broadcast([P, nb, P]),
                        op=mybir.AluOpType.is_equal,
                    )
                    # bank-interleaved tile order, block-outer: consecutive
                    # matmuls never hit the same psum region/bank.
                    torder = [t for r in range(4) for t in range(r, tpg, 4)]
                    last_of_bank = {}
                    maxb = int(blocks[g, c].max())
                    for b in range(maxb):
                        for t in torder:
                            if b < blocks[g, c, t]:
                                last_of_bank[t // 4] = (t, b)
                    started = set()
                    for b in range(maxb):
                        for t in torder:
                            if b >= blocks[g, c, t]:
                                continue
                            j = int(boff[bin_id(g, c, t)] - sb0) + b
                            bank = t // 4
                            start = c == 0 and b == 0 and bank not in started
                            if start:
                                started.add(bank)
                            nc.tensor.matmul(
                                out=pregion(g, t),
                                lhsT=sl[:, j, :],
                                rhs=mg[:, j, 0:D],
                                start=start,
                                stop=(
                                    c == nch - 1
                                    and last_of_bank[bank] == (t, b)
                                ),
                            )
                    step += 1
                sg = stg[g % 2]
                for k in range(bpg):
                    w = min(4, tpg - 4 * k)
                    bk = banks[(g % 2) * bpg + k]
                    nc.scalar.copy(
                        sg[:, 4 * k * D : (4 * k + w) * D], bk[:, : w * D]
                    )
                for t in range(tpg):
                    r0 = (g * tpg + t) * P
                    nc.sync.dma_start(
                        out=out[r0 : r0 + P, :], in_=sg[:, t * D : (t + 1) * D]
                    )

    for blk in nc.main_func.blocks:
        for ins in blk.instructions:
            if isinstance(ins, mybir.InstDMAGatherAnt):
                si = ins.sync_info
                if si and si.on_update:
                    name = si.on_update[0].ant_name
                    lane = int(name.split("_")[0][len("DMASW") :])
                    ins.queue_num = lane % nq
    nc.compile()
    return nc


def prep_core(src, rel, spans, caps, n_groups, tpg):
    """Bin one core's edges (src global, rel = tile*128+p core-relative slot)
    into the (group, chunk, tile) layout. Returns (idxT, dstT, cntT)."""
    nch = len(spans)
    blocks = np.asarray(caps)
    nblk = int(blocks.sum())
    nbins = n_groups * nch * tpg
    npair = (tpg + 1) // 2
    boff = np.zeros(nbins + 1, np.int64)
    boff[1:] = np.cumsum(blocks.reshape(-1))

    t = rel >> 7
    g = t // tpg
    ti = t % tpg
    chunk = spans[0][1]
    c = np.minimum(src // chunk, nch - 1)
    bases = np.array([b for b, s in spans], dtype=np.int64)
    sr = src - bases[c]

    binkey = (g * nch + c) * tpg + ti
    order = np.lexsort((sr, binkey))
    sr, relo, binkey = sr[order], rel[order], binkey[order]
    counts = np.bincount(binkey, minlength=nbins)
    capacity = blocks.reshape(-1) * P
    if (counts > capacity).any():
        raise ValueError("caps too small")
    starts = np.zeros(nbins, np.int64)
    starts[1:] = np.cumsum(counts)[:-1]
    pos = np.arange(len(sr)) - starts[binkey]
    slot = boff[binkey] * P + pos

    total = nblk * P
    srcflat = np.full(total, -1, np.int64)
    dstflat = np.full(total, 255, np.int64)  # 255 = stale (sel col 0)
    srcflat[slot] = sr
    dstflat[slot] = relo & 127

    # merged pair gathers: mid-bins pad with idx 0 (valid, sel-zeroed);
    # the final bin of each pair keeps -1 (trailing trim via count reg).
    cnt = np.zeros(n_groups * nch * npair, np.int64)
    for gg in range(n_groups):
        for cc in range(nch):
            for pi in range(npair):
                t0 = 2 * pi
                t1 = min(2 * pi + 1, tpg - 1)
                k0 = (gg * nch + cc) * tpg + t0
                k1 = (gg * nch + cc) * tpg + t1
                gi = (gg * nch + cc) * npair + pi
                if t1 > t0:
                    # pad bin t0's tail (idx 0) up to its capacity
                    s0, e0 = boff[k0] * P, (boff[k0] + blocks[gg, cc, t0]) * P
                    tail = srcflat[s0:e0]
                    tail[tail < 0] = 0
                    cnt[gi] = (e0 - s0) + counts[k1]
                    if counts[k1] == 0:
                        # need a valid final idx for the trim loop
                        srcflat[boff[k1] * P] = 0
                        cnt[gi] = (e0 - s0) + 1
                else:
                    cnt[gi] = counts[k0]
                    if counts[k0] == 0:
                        srcflat[boff[k0] * P] = 0
                        cnt[gi] = 1

    idxT = np.empty((16, total // 16), np.int16)
    seg = srcflat.reshape(-1, 16)
    idxT[:, :] = seg.T.reshape(16, total // 16)
    idxT = np.tile(idxT, (8, 1))
    dstT = np.ascontiguousarray(
        dstflat.reshape(nblk, P).T.astype(ml_dtypes.bfloat16)
    )
    cntT = np.tile(cnt.astype(np.int32), (P, 1))
    return idxT, dstT, cntT


def compute_caps(binned_counts):
    """binned_counts: [n_cores, n_groups, nch, tpg] -> blocks per bin
    (max over cores, ceil /128)."""
    mx = binned_counts.max(axis=0)
    return np.maximum(1, -(-mx // P)).astype(np.int64)


_cache = {}


def kernel(x, edge_index):
    TILES = 112
    TPG = 16
    CHUNK = 25000
    DESC_ROWS = int(os.environ.get("KERNEL_DESC_ROWS", "1"))
    NBUF_M = int(os.environ.get("KERNEL_NBUF_M", "3"))
    NQ = 4

    x = np.asarray(x, dtype=np.float32)
    edge_index = np.asarray(edge_index)
    src = edge_index[0].astype(np.int64)
    dst = edge_index[1].astype(np.int64)

    n_groups = TILES // TPG
    spans = make_chunks(N_NODES, CHUNK)
    nch = len(spans)

    node_slot = balance_nodes(src, dst, N_CORES, TILES, spans)
    eslot = node_slot[dst]
    ecore = eslot // (TILES * P)
    erel = eslot % (TILES * P)

    # per-core bin counts for caps
    cid = np.minimum(src // CHUNK, nch - 1)
    t = erel >> 7
    bk = ((ecore * n_groups + t // TPG) * nch + cid) * TPG + (t % TPG)
    bc = np.bincount(bk, minlength=N_CORES * n_groups * nch * TPG).reshape(
        N_CORES, n_groups, nch, TPG
    )
    caps = compute_caps(bc)

    key = (caps.tobytes(), n_groups, TPG, DESC_ROWS, NBUF_M)
    if key not in _cache:
        _cache[key] = build_program(
            spans, caps, n_groups, TPG, N_CORES, DESC_ROWS, NBUF_M, NQ
        )
    nc = _cache[key]

    xbf = np.zeros((N_NODES + 2 * DESC_ROWS, D), ml_dtypes.bfloat16)
    xbf[:N_NODES] = x.astype(ml_dtypes.bfloat16)
    iota = np.tile(
        np.arange(P, dtype=np.float32).astype(ml_dtypes.bfloat16), (P, 1)
    )
    in_maps = []
    for k in range(N_CORES):
        m = ecore == k
        idxT, dstT, cntT = prep_core(
            src[m], erel[m], spans, caps, n_groups, TPG
        )
        in_maps.append(
            {"xbf": xbf, "idxT": idxT, "dstT": dstT, "iota": iota, "cntT": cntT}
        )

    trace = bool(int(os.environ.get("KERNEL_TRACE", "0")))
    res = run_bass_kernel_spmd(
        nc, in_maps, core_ids=list(range(N_CORES)), trace=trace
    )
    if trace:
        kernel.last_results = res
    dev = np.stack([res.results[c]["out"] for c in range(N_CORES)])  # [8, T*128, D]
    full = np.empty((N_NODES, D), np.float32)
    full[:] = dev.reshape(N_CORES * TILES * P, D)[node_slot]
    return np.ascontiguousarray(full)


# revision 13
# speedup vs baseline: 1.2619x; 1.0358x over previous
"""GNN message passing (gather + scatter-add) on 8 trn2 NeuronCores, v4.

Strategy (dst-sharded, gather via InstDMAGatherAnt on 4 SWDGE queues):
  * The host ASSIGNS dst nodes to (core, tile, partition) slots with a
    load balancer (serpentine deal on degree + per-(tile,chunk) repair
    swaps) so every (group, chunk, tile) bin has ~equal edge count.
    This shrinks the SPMD-uniform bin capacities to ~mean (the baseline
    paid max-over-784-Poisson-bins), cutting msg/sel SBUF and blocks.
  * 112 tiles of 128 dst slots per core (TPG=16, 7 groups; 4 PSUM banks
    per group, 2 groups in flight = 8 banks). x stays in HBM as bf16;
    edges are binned by (group, chunk=25000 src rows, tile); each bin is
    gathered by ONE dma_gather (chunk-relative int16 idx), bins for
    adjacent tile pairs are merged into one gather (<=1024 ring descs)
    with mid-stream idx-0 pads (dst code 255 => sel column 0) and
    trailing -1 (trimmed by ucode via the per-core count register).
  * Descriptors can be 512B (DESC_ROWS=2: each desc fetches rows
    [src, src+2), the matmul reads cols 0:128) - measured ~16% faster
    per descriptor than 256B on the SWDGE queue pipeline.
  * One DVE is_equal per (group, chunk) step builds one-hot sel planes;
    one matmul per 128-slot block accumulates psum[dst,f] += sel^T@msg.
    Stale slots (count < capacity) keep old finite bf16 data and get
    sel 0. PSUM start/stop once per bank per group phase.
  * psum -> SBUF via scalar engine (whole-bank copies), HWDGE DMA out;
    host un-permutes rows via the balancer's node map.
No collective needed; each core owns its output rows.
"""

import os
import sys

import numpy as np
import ml_dtypes

for _p in ("/opt/trn_rl_repo",):
    if _p not in sys.path:
        sys.path.insert(0, _p)

import bass_rust  # noqa: E402
from concourse import bass, mybir, tile, bacc, library_config  # noqa: E402
from concourse.bass_utils import run_bass_kernel_spmd  # noqa: E402

P = 128
D = 128
N_NODES = 100000
N_CORES = 8

NBUF_S = 2  # sel buffers


def make_chunks(n_src, chunk):
    spans = []
    b = 0
    while b < n_src:
        s = min(chunk, n_src - b)
        spans.append((b, s))
        b += s
    return spans


def balance_nodes(src, dst, n_cores, tiles, spans):
    """Assign each dst node a (core, tile, partition) slot, balancing the
    per-(tile, chunk) edge counts. Returns node_slot [N] int64 encoding
    core*tiles*128 + tile*128 + p, with every tile holding <=128 nodes."""
    n = N_NODES
    nch = len(spans)
    ntile = n_cores * tiles
    # per-node per-chunk degree
    cid = np.minimum(src // spans[0][1], nch - 1)
    degc = np.zeros((nch, n), np.int32)
    for c in range(nch):
        degc[c] = np.bincount(dst[cid == c], minlength=n)
    deg = degc.sum(axis=0)

    # serpentine deal on total degree: round r gives one node to each tile,
    # pairing heaviest remaining nodes with lightest tiles.
    order = np.argsort(-deg, kind="stable")
    loads = np.zeros(ntile, np.int64)
    fill = np.zeros(ntile, np.int32)
    assign = np.empty(n, np.int64)
    pos = 0
    while pos < n:
        batch = order[pos : pos + ntile]
        tl = np.argsort(loads, kind="stable")[: len(batch)]
        assign[batch] = tl
        loads[tl] += deg[batch]
        fill[tl] += 1
        pos += len(batch)

    # repair pass: per-(tile, chunk) loads; swap high-deg nodes out of
    # overloaded bins into the lightest tiles (matched by total degree).
    cl = np.zeros((ntile, nch), np.int64)
    for c in range(nch):
        np.add.at(cl[:, c], assign, degc[c])
    for _ in range(200):
        worst = np.unravel_index(np.argmax(cl), cl.shape)
        t0, c0 = int(worst[0]), int(worst[1])
        lim = cl.max(axis=1).mean() + 8
        if cl[t0, c0] <= lim:
            break
        cand = np.flatnonzero(assign == t0)
        mover = cand[np.argmax(degc[c0, cand])]
        t1 = int(np.argmin(cl[:, c0] + (fill >= 128) * (1 << 40)))
        # swap mover with a node in t1 of similar total degree but low c0 deg
        cand1 = np.flatnonzero(assign == t1)
        recv = cand1[np.argmin(degc[c0, cand1].astype(np.int64) * (1 << 20) - deg[cand1])]
        assign[mover], assign[recv] = t1, t0
        cl[t0] += degc[:, recv] - degc[:, mover]
        cl[t1] += degc[:, mover] - degc[:, recv]

    # partition index within tile
    order2 = np.argsort(assign, kind="stable")
    idx_in_tile = np.empty(n, np.int64)
    start = 0
    counts = np.bincount(assign, minlength=ntile)
    assert counts.max() <= 128
    off = np.concatenate([[0], np.cumsum(counts)])
    ranks = np.arange(n) - off[assign[order2]]
    idx_in_tile[order2] = ranks
    node_slot = assign * P + idx_in_tile
    return node_slot  # global slot id: (core*tiles + tile)*128 + p


def build_program(spans, caps, n_groups, tpg, num_devices, desc_rows, nbuf_m, nq):
    """caps: int array [n_groups, nch, tpg] = blocks per bin (uniform across
    cores). Gathers merge adjacent tile pairs. Output rows: n_groups*tpg*128."""
    nch = len(spans)
    E = D * desc_rows  # gathered elems per slot
    blocks = np.asarray(caps)  # [g][c][t]
    nblk = int(blocks.sum())
    step_blocks = blocks.sum(axis=2)  # [g][c]
    max_nb = int(step_blocks.max())
    nbins = n_groups * nch * tpg
    # merged gathers: pairs of adjacent tiles
    npair = (tpg + 1) // 2
    ngath = n_groups * nch * npair

    nc = bacc.Bacc(
        "TRN2",
        target_bir_lowering=False,
        debug=False,
        num_devices=num_devices,
        num_swdge_queues=nq,
        # ring of 2048 descriptors per SWDGE queue: two 1024-desc gathers in
        # flight per queue (deeper gen/drain pipelining, measured ~15% faster
        # per descriptor than the default 1024-desc ring).
        dynamic_dma_scratch_size=32768,
    )
    n_src = spans[-1][0] + spans[-1][1]
    xbf = nc.dram_tensor(
        "xbf", [n_src + 2 * desc_rows, D], mybir.dt.bfloat16, kind="ExternalInput"
    ).ap()
    ncol = nblk * P // 16
    idxT = nc.dram_tensor("idxT", [P, ncol], mybir.dt.int16, kind="ExternalInput").ap()
    dstT = nc.dram_tensor(
        "dstT", [P, nblk], mybir.dt.bfloat16, kind="ExternalInput"
    ).ap()
    iota = nc.dram_tensor(
        "iota", [P, P], mybir.dt.bfloat16, kind="ExternalInput"
    ).ap()
    out = nc.dram_tensor(
        "out", [n_groups * tpg * P, D], mybir.dt.float32, kind="ExternalOutput"
    ).ap()

    # slot offset (in blocks) of each bin, ordered (g, c, t)
    boff = np.zeros(nbins + 1, np.int64)
    boff[1:] = np.cumsum(blocks.reshape(-1))

    def bin_id(g, c, t):
        return (g * nch + c) * tpg + t

    with tile.TileContext(nc) as tc:
        with tc.tile_pool(name="sb", bufs=1) as pool, tc.tile_pool(
            name="ps", bufs=1, space="PSUM"
        ) as psp:
            idxs = pool.tile([P, ncol], mybir.dt.int16)
            dsts = pool.tile([P, nblk], mybir.dt.bfloat16)
            iot = pool.tile([P, P], mybir.dt.bfloat16)
            nc.sync.dma_start(out=idxs[:], in_=idxT[:])
            nc.sync.dma_start(out=dsts[:], in_=dstT[:])
            nc.sync.dma_start(out=iot[:], in_=iota[:])
            nc.gpsimd.load_library(library_config.mlp)

            msg = [
                pool.tile([P, max_nb, E], mybir.dt.bfloat16, name=f"msg{i}")
                for i in range(nbuf_m)
            ]
            sel = [
                pool.tile([P, max_nb, P], mybir.dt.bfloat16, name=f"sel{i}")
                for i in range(NBUF_S)
            ]
            stg = [
                pool.tile([P, tpg * D], mybir.dt.float32, name=f"stg{i}")
                for i in range(2)
            ]
            # no msg memsets: the first nbuf_m steps gather at FULL capacity
            # (host pads with idx 0 / dst 255), so stale slots always hold
            # finite bf16 data from a real row thereafter.
            bpg = -(-tpg // 4)  # banks per group
            assert 2 * bpg <= 8
            banks = [
                psp.tile([P, 4 * D], dtype=mybir.dt.float32, space="PSUM", name=f"psb{j}")
                for j in range(2 * bpg)
            ]

            def pregion(g, t):
                bk = banks[(g % 2) * bpg + t // 4]
                return bk[:, (t % 4) * D : (t % 4 + 1) * D]

            # one register per distinct gather capacity, set once: the ucode's
            # trailing -1 trim recovers each core's actual count, so no
            # per-gather reg_load is needed.

            capregs = {}
            for g in range(n_groups):
                for c in range(nch):
                    for pi in range(npair):
                        t0, t1 = 2 * pi, min(2 * pi + 1, tpg - 1)
                        ns = int(
                            boff[bin_id(g, c, t1)]
                            + blocks[g, c, t1]
                            - boff[bin_id(g, c, t0)]
                        ) * P
                        if ns not in capregs:
                            capregs[ns] = nc.gpsimd.alloc_register(f"cap{ns}")
            for ns, r in capregs.items():
                nc.gpsimd.reg_mov(r, ns)

            step = 0
            gq = 0
            for g in range(n_groups):
                for c in range(nch):
                    km = step % nbuf_m
                    ks = step % NBUF_S
                    mg, sl = msg[km], sel[ks]
                    base, span = spans[c]
                    nb = int(step_blocks[g, c])
                    sb0 = boff[bin_id(g, c, 0)]  # first block of this step
                    inap = xbf[base : base + span + 2 * desc_rows, :]
                    if desc_rows > 1:
                        # overlapping window view: row i -> elems [i*D, i*D+E)
                        inap = inap.copy()
                        inap.ap = bass_rust.VecI64Pair(
                            [(D, span + desc_rows), (1, E)]
                        )
                    for pi in range(npair):
                        t0 = 2 * pi
                        t1 = min(2 * pi + 1, tpg - 1)
                        b0 = boff[bin_id(g, c, t0)]
                        bend = boff[bin_id(g, c, t1)] + blocks[g, c, t1]
                        nslot = int(bend - b0) * P
                        coloff = int(b0) * P // 16
                        nc.gpsimd.dma_gather(
                            mg[:, int(b0 - sb0) : int(bend - sb0), :],
                            inap,
                            idxs[:, coloff : coloff + nslot // 16],
                            nslot,
                            capregs[nslot],
                            E,
                            elem_step=D,
                            queue_num=gq % nq,
                        )
                        gq += 1
                    nc.vector.tensor_tensor(
                        out=sl[:, :nb, :],
                        in0=dsts[:, int(sb0) : int(sb0 + nb)][:, :, None].to_broadcast(
                            [P, nb, P]
                        ),
                        in1=iot[:, None, :].to_broadcast([P, nb, P]),
                        op=mybir.AluOpType.is_equal,
                    )
                    # bank-interleaved tile order, block-outer: consecutive
                    # matmuls never hit the same psum region/bank.
                    torder = [t for r in range(4) for t in range(r, tpg, 4)]
                    last_of_bank = {}
                    maxb = int(blocks[g, c].max())
                    for b in range(maxb):
                        for t in torder:
                            if b < blocks[g, c, t]:
                                last_of_bank[t // 4] = (t, b)
                    started = set()
                    for b in range(maxb):
                        for t in torder:
                            if b >= blocks[g, c, t]:
                                continue
                            j = int(boff[bin_id(g, c, t)] - sb0) + b
                            bank = t // 4
                            start = c == 0 and b == 0 and bank not in started
                            if start:
                                started.add(bank)
                            nc.tensor.matmul(
                                out=pregion(g, t),
                                lhsT=sl[:, j, :],
                                rhs=mg[:, j, 0:D],
                                start=start,
                                stop=(
                                    c == nch - 1
                                    and last_of_bank[bank] == (t, b)
                                ),
                            )
                    step += 1
                sg = stg[g % 2]
                for k in range(bpg):
                    w = min(4, tpg - 4 * k)
                    bk = banks[(g % 2) * bpg + k]
                    nc.scalar.copy(
                        sg[:, 4 * k * D : (4 * k + w) * D], bk[:, : w * D]
                    )
                for t in range(tpg):
                    r0 = (g * tpg + t) * P
                    nc.sync.dma_start(
                        out=out[r0 : r0 + P, :], in_=sg[:, t * D : (t + 1) * D]
                    )

    for blk in nc.main_func.blocks:
        for ins in blk.instructions:
            if isinstance(ins, mybir.InstDMAGatherAnt):
                si = ins.sync_info
                if si and si.on_update:
                    name = si.on_update[0].ant_name
                    lane = int(name.split("_")[0][len("DMASW") :])
                    ins.queue_num = lane % nq
    nc.compile()
    return nc


def prep_core(src, rel, spans, caps, n_groups, tpg, nbuf_m=3):
    """Bin one core's edges (src global, rel = tile*128+p core-relative slot)
    into the (group, chunk, tile) layout. Returns (idxT, dstT).

    Gathers pass the (compile-time) capacity register; per-core counts are
    recovered by the ucode's trailing -1 trim. Mid-pads (first bin of each
    merged pair, and ALL pads in the first nbuf_m steps so msg buffers get
    fully initialized without memsets) are idx 0 with dst code 255."""
    nch = len(spans)
    blocks = np.asarray(caps)
    nblk = int(blocks.sum())
    nbins = n_groups * nch * tpg
    boff = np.zeros(nbins + 1, np.int64)
    boff[1:] = np.cumsum(blocks.reshape(-1))

    t = rel >> 7
    g = t // tpg
    ti = t % tpg
    chunk = spans[0][1]
    c = np.minimum(src // chunk, nch - 1)
    bases = np.array([b for b, s in spans], dtype=np.int64)
    sr = src - bases[c]

    binkey = (g * nch + c) * tpg + ti
    order = np.lexsort((sr, binkey))
    sr, relo, binkey = sr[order], rel[order], binkey[order]
    counts = np.bincount(binkey, minlength=nbins)
    capacity = blocks.reshape(-1) * P
    if (counts > capacity).any():
        raise ValueError("caps too small")
    starts = np.zeros(nbins, np.int64)
    starts[1:] = np.cumsum(counts)[:-1]
    pos = np.arange(len(sr)) - starts[binkey]
    slot = boff[binkey] * P + pos

    total = nblk * P
    srcflat = np.full(total, -1, np.int64)
    dstflat = np.full(total, 255, np.int64)  # 255 = stale/pad (sel col 0)
    srcflat[slot] = sr
    dstflat[slot] = relo & 127

    # pad everything except each merged pair's trailing bin with idx 0
    kk = np.arange(nbins)
    stepk = kk // tpg
    tk = kk % tpg
    is_last_of_pair = (tk % 2 == 1) | (tk == tpg - 1)
    first_use = stepk < nbuf_m
    pad0 = ~is_last_of_pair | first_use
    binf = np.repeat(np.arange(nbins), blocks.reshape(-1) * P)
    fill = (srcflat < 0) & pad0[binf]
    srcflat[fill] = 0

    idxT = np.empty((16, total // 16), np.int16)
    seg = srcflat.reshape(-1, 16)
    idxT[:, :] = seg.T.reshape(16, total // 16)
    idxT = np.tile(idxT, (8, 1))
    dstT = np.ascontiguousarray(
        dstflat.reshape(nblk, P).T.astype(ml_dtypes.bfloat16)
    )
    return idxT, dstT


def compute_caps(binned_counts):
    """binned_counts: [n_cores, n_groups, nch, tpg] -> blocks per bin
    (max over cores, ceil /128)."""
    mx = binned_counts.max(axis=0)
    return np.maximum(1, -(-mx // P)).astype(np.int64)


_cache = {}


def kernel(x, edge_index):
    TILES = 112
    TPG = 16
    CHUNK = 25000
    DESC_ROWS = int(os.environ.get("KERNEL_DESC_ROWS", "1"))
    NBUF_M = int(os.environ.get("KERNEL_NBUF_M", "3"))
    NQ = 4

    x = np.asarray(x, dtype=np.float32)
    edge_index = np.asarray(edge_index)
    src = edge_index[0].astype(np.int64)
    dst = edge_index[1].astype(np.int64)

    n_groups = TILES // TPG
    spans = make_chunks(N_NODES, CHUNK)
    nch = len(spans)

    node_slot = balance_nodes(src, dst, N_CORES, TILES, spans)
    eslot = node_slot[dst]
    ecore = eslot // (TILES * P)
    erel = eslot % (TILES * P)

    # per-core bin counts for caps
    cid = np.minimum(src // CHUNK, nch - 1)
    t = erel >> 7
    bk = ((ecore * n_groups + t // TPG) * nch + cid) * TPG + (t % TPG)
    bc = np.bincount(bk, minlength=N_CORES * n_groups * nch * TPG).reshape(
        N_CORES, n_groups, nch, TPG
    )
    caps = compute_caps(bc)

    key = (caps.tobytes(), n_groups, TPG, DESC_ROWS, NBUF_M)
    if key not in _cache:
        _cache[key] = build_program(
            spans, caps, n_groups, TPG, N_CORES, DESC_ROWS, NBUF_M, NQ
        )
    nc = _cache[key]

    xbf = np.zeros((N_NODES + 2 * DESC_ROWS, D), ml_dtypes.bfloat16)
    xbf[:N_NODES] = x.astype(ml_dtypes.bfloat16)
    iota = np.tile(
        np.arange(P, dtype=np.float32).astype(ml_dtypes.bfloat16), (P, 1)
    )
    in_maps = []
    for k in range(N_CORES):
        m = ecore == k
        idxT, dstT = prep_core(
            src[m], erel[m], spans, caps, n_groups, TPG, NBUF_M
        )
        in_maps.append({"xbf": xbf, "idxT": idxT, "dstT": dstT, "iota": iota})

    trace = bool(int(os.environ.get("KERNEL_TRACE", "0")))
    res = run_bass_kernel_spmd(
        nc, in_maps, core_ids=list(range(N_CORES)), trace=trace
    )
    if trace:
        kernel.last_results = res
    dev = np.stack([res.results[c]["out"] for c in range(N_CORES)])  # [8, T*128, D]
    full = np.empty((N_NODES, D), np.float32)
    full[:] = dev.reshape(N_CORES * TILES * P, D)[node_slot]
    return np.ascontiguousarray(full)


# revision 14
# speedup vs baseline: 1.2866x; 1.0196x over previous
"""GNN message passing (gather + scatter-add) on 8 trn2 NeuronCores, v4.

Strategy (dst-sharded, gather via InstDMAGatherAnt on 4 SWDGE queues):
  * The host ASSIGNS dst nodes to (core, tile, partition) slots with a
    load balancer (serpentine deal on degree + per-(tile,chunk) repair
    swaps) so every (group, chunk, tile) bin has ~equal edge count.
    This shrinks the SPMD-uniform bin capacities to ~mean (the baseline
    paid max-over-784-Poisson-bins), cutting msg/sel SBUF and blocks.
  * 112 tiles of 128 dst slots per core (TPG=16, 7 groups; 4 PSUM banks
    per group, 2 groups in flight = 8 banks). x stays in HBM as bf16;
    edges are binned by (group, chunk=25000 src rows, tile); each bin is
    gathered by ONE dma_gather (chunk-relative int16 idx), bins for
    adjacent tile pairs are merged into one gather (<=1024 ring descs)
    with mid-stream idx-0 pads (dst code 255 => sel column 0) and
    trailing -1 (trimmed by ucode via the per-core count register).
  * Descriptors can be 512B (DESC_ROWS=2: each desc fetches rows
    [src, src+2), the matmul reads cols 0:128) - measured ~16% faster
    per descriptor than 256B on the SWDGE queue pipeline.
  * One DVE is_equal per (group, chunk) step builds one-hot sel planes;
    one matmul per 128-slot block accumulates psum[dst,f] += sel^T@msg.
    Stale slots (count < capacity) keep old finite bf16 data and get
    sel 0. PSUM start/stop once per bank per group phase.
  * psum -> SBUF via scalar engine (whole-bank copies), HWDGE DMA out;
    host un-permutes rows via the balancer's node map.
No collective needed; each core owns its output rows.
"""

import os
import sys

import numpy as np
import ml_dtypes

for _p in ("/opt/trn_rl_repo",):
    if _p not in sys.path:
        sys.path.insert(0, _p)

import bass_rust  # noqa: E402
from concourse import bass, mybir, tile, bacc, library_config  # noqa: E402
from concourse.bass_utils import run_bass_kernel_spmd  # noqa: E402

P = 128
D = 128
N_NODES = 100000
N_CORES = 8

NBUF_S = 2  # sel buffers


def make_chunks(n_src, chunk):
    spans = []
    b = 0
    while b < n_src:
        s = min(chunk, n_src - b)
        spans.append((b, s))
        b += s
    return spans


def balance_nodes(src, dst, n_cores, tiles, spans):
    """Assign each dst node a (core, tile, partition) slot, balancing the
    per-(tile, chunk) edge counts. Returns node_slot [N] int64 encoding
    core*tiles*128 + tile*128 + p, with every tile holding <=128 nodes."""
    n = N_NODES
    nch = len(spans)
    ntile = n_cores * tiles
    # per-node per-chunk degree
    cid = np.minimum(src // spans[0][1], nch - 1)
    degc = np.zeros((nch, n), np.int32)
    for c in range(nch):
        degc[c] = np.bincount(dst[cid == c], minlength=n)
    deg = degc.sum(axis=0)

    # serpentine deal on total degree: round r gives one node to each tile,
    # pairing heaviest remaining nodes with lightest tiles.
    order = np.argsort(-deg, kind="stable")
    loads = np.zeros(ntile, np.int64)
    fill = np.zeros(ntile, np.int32)
    assign = np.empty(n, np.int64)
    pos = 0
    while pos < n:
        batch = order[pos : pos + ntile]
        tl = np.argsort(loads, kind="stable")[: len(batch)]
        assign[batch] = tl
        loads[tl] += deg[batch]
        fill[tl] += 1
        pos += len(batch)

    # repair pass: per-(tile, chunk) loads; swap high-deg nodes out of
    # overloaded bins into the lightest tiles (matched by total degree).
    cl = np.zeros((ntile, nch), np.int64)
    for c in range(nch):
        np.add.at(cl[:, c], assign, degc[c])
    for _ in range(200):
        worst = np.unravel_index(np.argmax(cl), cl.shape)
        t0, c0 = int(worst[0]), int(worst[1])
        lim = cl.max(axis=1).mean() + 8
        if cl[t0, c0] <= lim:
            break
        cand = np.flatnonzero(assign == t0)
        mover = cand[np.argmax(degc[c0, cand])]
        t1 = int(np.argmin(cl[:, c0] + (fill >= 128) * (1 << 40)))
        # swap mover with a node in t1 of similar total degree but low c0 deg
        cand1 = np.flatnonzero(assign == t1)
        recv = cand1[np.argmin(degc[c0, cand1].astype(np.int64) * (1 << 20) - deg[cand1])]
        assign[mover], assign[recv] = t1, t0
        cl[t0] += degc[:, recv] - degc[:, mover]
        cl[t1] += degc[:, mover] - degc[:, recv]

    # partition index within tile
    order2 = np.argsort(assign, kind="stable")
    idx_in_tile = np.empty(n, np.int64)
    start = 0
    counts = np.bincount(assign, minlength=ntile)
    assert counts.max() <= 128
    off = np.concatenate([[0], np.cumsum(counts)])
    ranks = np.arange(n) - off[assign[order2]]
    idx_in_tile[order2] = ranks
    node_slot = assign * P + idx_in_tile
    return node_slot  # global slot id: (core*tiles + tile)*128 + p


def build_program(spans, caps, n_groups, tpg, num_devices, desc_rows, nbuf_m, nq):
    """caps: int array [n_groups, nch, tpg] = blocks per bin (uniform across
    cores). Gathers merge adjacent tile pairs. Output rows: n_groups*tpg*128."""
    nch = len(spans)
    E = D * desc_rows  # gathered elems per slot
    blocks = np.asarray(caps)  # [g][c][t]
    nblk = int(blocks.sum())
    step_blocks = blocks.sum(axis=2)  # [g][c]
    max_nb = int(step_blocks.max())
    nbins = n_groups * nch * tpg
    # merged gathers: pairs of adjacent tiles
    npair = (tpg + 1) // 2
    ngath = n_groups * nch * npair

    nc = bacc.Bacc(
        "TRN2",
        target_bir_lowering=False,
        debug=False,
        num_devices=num_devices,
        num_swdge_queues=nq,
        # ring of 2048 descriptors per SWDGE queue: two 1024-desc gathers in
        # flight per queue (deeper gen/drain pipelining, measured ~15% faster
        # per descriptor than the default 1024-desc ring).
        dynamic_dma_scratch_size=32768,
    )
    n_src = spans[-1][0] + spans[-1][1]
    xbf = nc.dram_tensor(
        "xbf", [n_src + 2 * desc_rows, D], mybir.dt.bfloat16, kind="ExternalInput"
    ).ap()
    ncol = nblk * P // 16
    idxT = nc.dram_tensor("idxT", [P, ncol], mybir.dt.int16, kind="ExternalInput").ap()
    dstT = nc.dram_tensor(
        "dstT", [P, nblk], mybir.dt.bfloat16, kind="ExternalInput"
    ).ap()
    iota = nc.dram_tensor(
        "iota", [P, P], mybir.dt.bfloat16, kind="ExternalInput"
    ).ap()
    out = nc.dram_tensor(
        "out", [n_groups * tpg * P, D], mybir.dt.float32, kind="ExternalOutput"
    ).ap()

    # slot offset (in blocks) of each bin, ordered (g, c, t)
    boff = np.zeros(nbins + 1, np.int64)
    boff[1:] = np.cumsum(blocks.reshape(-1))

    def bin_id(g, c, t):
        return (g * nch + c) * tpg + t

    with tile.TileContext(nc) as tc:
        with tc.tile_pool(name="sb", bufs=1) as pool, tc.tile_pool(
            name="ps", bufs=1, space="PSUM"
        ) as psp:
            idxs = pool.tile([P, ncol], mybir.dt.int16)
            dsts = pool.tile([P, nblk], mybir.dt.bfloat16)
            iot = pool.tile([P, P], mybir.dt.bfloat16)
            # per-step idx slices: the first gathers only wait on their own
            # slice, not the whole 4+ MB index upload
            for g in range(n_groups):
                for c in range(nch):
                    s0 = int(boff[bin_id(g, c, 0)]) * P // 16
                    s1 = int(boff[bin_id(g, c, tpg - 1)] + blocks[g, c, tpg - 1]) * P // 16
                    nc.sync.dma_start(out=idxs[:, s0:s1], in_=idxT[:, s0:s1])
            nc.sync.dma_start(out=dsts[:], in_=dstT[:])
            nc.sync.dma_start(out=iot[:], in_=iota[:])
            nc.gpsimd.load_library(library_config.mlp)

            msg = [
                pool.tile([P, max_nb, E], mybir.dt.bfloat16, name=f"msg{i}")
                for i in range(nbuf_m)
            ]
            sel = [
                pool.tile([P, max_nb, P], mybir.dt.bfloat16, name=f"sel{i}")
                for i in range(NBUF_S)
            ]
            stg = [
                pool.tile([P, tpg * D], mybir.dt.float32, name=f"stg{i}")
                for i in range(2)
            ]
            # no msg memsets: the first nbuf_m steps gather at FULL capacity
            # (host pads with idx 0 / dst 255), so stale slots always hold
            # finite bf16 data from a real row thereafter.
            bpg = -(-tpg // 4)  # banks per group
            assert 2 * bpg <= 8
            banks = [
                psp.tile([P, 4 * D], dtype=mybir.dt.float32, space="PSUM", name=f"psb{j}")
                for j in range(2 * bpg)
            ]

            def pregion(g, t):
                bk = banks[(g % 2) * bpg + t // 4]
                return bk[:, (t % 4) * D : (t % 4 + 1) * D]

            # one register per distinct gather capacity, set once: the ucode's
            # trailing -1 trim recovers each core's actual count, so no
            # per-gather reg_load is needed.

            capregs = {}
            for g in range(n_groups):
                for c in range(nch):
                    for pi in range(npair):
                        t0, t1 = 2 * pi, min(2 * pi + 1, tpg - 1)
                        ns = int(
                            boff[bin_id(g, c, t1)]
                            + blocks[g, c, t1]
                            - boff[bin_id(g, c, t0)]
                        ) * P
                        if ns not in capregs:
                            capregs[ns] = nc.gpsimd.alloc_register(f"cap{ns}")
            for ns, r in capregs.items():
                nc.gpsimd.reg_mov(r, ns)

            step = 0
            gq = 0
            for g in range(n_groups):
                for c in range(nch):
                    km = step % nbuf_m
                    ks = step % NBUF_S
                    mg, sl = msg[km], sel[ks]
                    base, span = spans[c]
                    nb = int(step_blocks[g, c])
                    sb0 = boff[bin_id(g, c, 0)]  # first block of this step
                    inap = xbf[base : base + span + 2 * desc_rows, :]
                    if desc_rows > 1:
                        # overlapping window view: row i -> elems [i*D, i*D+E)
                        inap = inap.copy()
                        inap.ap = bass_rust.VecI64Pair(
                            [(D, span + desc_rows), (1, E)]
                        )
                    for pi in range(npair):
                        t0 = 2 * pi
                        t1 = min(2 * pi + 1, tpg - 1)
                        b0 = boff[bin_id(g, c, t0)]
                        bend = boff[bin_id(g, c, t1)] + blocks[g, c, t1]
                        nslot = int(bend - b0) * P
                        coloff = int(b0) * P // 16
                        nc.gpsimd.dma_gather(
                            mg[:, int(b0 - sb0) : int(bend - sb0), :],
                            inap,
                            idxs[:, coloff : coloff + nslot // 16],
                            nslot,
                            capregs[nslot],
                            E,
                            elem_step=D,
                            queue_num=gq % nq,
                        )
                        gq += 1
                    nc.vector.tensor_tensor(
                        out=sl[:, :nb, :],
                        in0=dsts[:, int(sb0) : int(sb0 + nb)][:, :, None].to_broadcast(
                            [P, nb, P]
                        ),
                        in1=iot[:, None, :].to_broadcast([P, nb, P]),
                        op=mybir.AluOpType.is_equal,
                    )
                    # bank-interleaved tile order, block-outer: consecutive
                    # matmuls never hit the same psum region/bank.
                    torder = [t for r in range(4) for t in range(r, tpg, 4)]
                    last_of_bank = {}
                    maxb = int(blocks[g, c].max())
                    for b in range(maxb):
                        for t in torder:
                            if b < blocks[g, c, t]:
                                last_of_bank[t // 4] = (t, b)
                    started = set()
                    for b in range(maxb):
                        for t in torder:
                            if b >= blocks[g, c, t]:
                                continue
                            j = int(boff[bin_id(g, c, t)] - sb0) + b
                            bank = t // 4
                            start = c == 0 and b == 0 and bank not in started
                            if start:
                                started.add(bank)
                            nc.tensor.matmul(
                                out=pregion(g, t),
                                lhsT=sl[:, j, :],
                                rhs=mg[:, j, 0:D],
                                start=start,
                                stop=(
                                    c == nch - 1
                                    and last_of_bank[bank] == (t, b)
                                ),
                            )
                    step += 1
                sg = stg[g % 2]
                for k in range(bpg):
                    w = min(4, tpg - 4 * k)
                    bk = banks[(g % 2) * bpg + k]
                    nc.scalar.copy(
                        sg[:, 4 * k * D : (4 * k + w) * D], bk[:, : w * D]
                    )
                for t in range(tpg):
                    r0 = (g * tpg + t) * P
                    nc.sync.dma_start(
                        out=out[r0 : r0 + P, :], in_=sg[:, t * D : (t + 1) * D]
                    )

    for blk in nc.main_func.blocks:
        for ins in blk.instructions:
            if isinstance(ins, mybir.InstDMAGatherAnt):
                si = ins.sync_info
                if si and si.on_update:
                    name = si.on_update[0].ant_name
                    lane = int(name.split("_")[0][len("DMASW") :])
                    ins.queue_num = lane % nq
    nc.compile()
    return nc


def prep_core(src, rel, spans, caps, n_groups, tpg, nbuf_m=3):
    """Bin one core's edges (src global, rel = tile*128+p core-relative slot)
    into the (group, chunk, tile) layout. Returns (idxT, dstT).

    Gathers pass the (compile-time) capacity register; per-core counts are
    recovered by the ucode's trailing -1 trim. Mid-pads (first bin of each
    merged pair, and ALL pads in the first nbuf_m steps so msg buffers get
    fully initialized without memsets) are idx 0 with dst code 255."""
    nch = len(spans)
    blocks = np.asarray(caps)
    nblk = int(blocks.sum())
    nbins = n_groups * nch * tpg
    boff = np.zeros(nbins + 1, np.int64)
    boff[1:] = np.cumsum(blocks.reshape(-1))

    t = rel >> 7
    g = t // tpg
    ti = t % tpg
    chunk = spans[0][1]
    c = np.minimum(src // chunk, nch - 1)
    bases = np.array([b for b, s in spans], dtype=np.int64)
    sr = src - bases[c]

    binkey = (g * nch + c) * tpg + ti
    order = np.lexsort((sr, binkey))
    sr, relo, binkey = sr[order], rel[order], binkey[order]
    counts = np.bincount(binkey, minlength=nbins)
    capacity = blocks.reshape(-1) * P
    if (counts > capacity).any():
        raise ValueError("caps too small")
    starts = np.zeros(nbins, np.int64)
    starts[1:] = np.cumsum(counts)[:-1]
    pos = np.arange(len(sr)) - starts[binkey]
    slot = boff[binkey] * P + pos

    total = nblk * P
    srcflat = np.full(total, -1, np.int64)
    dstflat = np.full(total, 255, np.int64)  # 255 = stale/pad (sel col 0)
    srcflat[slot] = sr
    dstflat[slot] = relo & 127

    # pad everything except each merged pair's trailing bin with idx 0
    kk = np.arange(nbins)
    stepk = kk // tpg
    tk = kk % tpg
    is_last_of_pair = (tk % 2 == 1) | (tk == tpg - 1)
    first_use = stepk < nbuf_m
    pad0 = ~is_last_of_pair | first_use
    binf = np.repeat(np.arange(nbins), blocks.reshape(-1) * P)
    fill = (srcflat < 0) & pad0[binf]
    srcflat[fill] = 0

    idxT = np.empty((16, total // 16), np.int16)
    seg = srcflat.reshape(-1, 16)
    idxT[:, :] = seg.T.reshape(16, total // 16)
    idxT = np.tile(idxT, (8, 1))
    dstT = np.ascontiguousarray(
        dstflat.reshape(nblk, P).T.astype(ml_dtypes.bfloat16)
    )
    return idxT, dstT


def compute_caps(binned_counts):
    """binned_counts: [n_cores, n_groups, nch, tpg] -> blocks per bin
    (max over cores, ceil /128)."""
    mx = binned_counts.max(axis=0)
    return np.maximum(1, -(-mx // P)).astype(np.int64)


_cache = {}


def kernel(x, edge_index):
    TILES = 112
    TPG = 16
    CHUNK = 25000
    DESC_ROWS = int(os.environ.get("KERNEL_DESC_ROWS", "1"))
    NBUF_M = int(os.environ.get("KERNEL_NBUF_M", "3"))
    NQ = 4

    x = np.asarray(x, dtype=np.float32)
    edge_index = np.asarray(edge_index)
    src = edge_index[0].astype(np.int64)
    dst = edge_index[1].astype(np.int64)

    n_groups = TILES // TPG
    spans = make_chunks(N_NODES, CHUNK)
    nch = len(spans)

    node_slot = balance_nodes(src, dst, N_CORES, TILES, spans)
    eslot = node_slot[dst]
    ecore = eslot // (TILES * P)
    erel = eslot % (TILES * P)

    # per-core bin counts for caps
    cid = np.minimum(src // CHUNK, nch - 1)
    t = erel >> 7
    bk = ((ecore * n_groups + t // TPG) * nch + cid) * TPG + (t % TPG)
    bc = np.bincount(bk, minlength=N_CORES * n_groups * nch * TPG).reshape(
        N_CORES, n_groups, nch, TPG
    )
    caps = compute_caps(bc)

    key = (caps.tobytes(), n_groups, TPG, DESC_ROWS, NBUF_M)
    if key not in _cache:
        _cache[key] = build_program(
            spans, caps, n_groups, TPG, N_CORES, DESC_ROWS, NBUF_M, NQ
        )
    nc = _cache[key]

    xbf = np.zeros((N_NODES + 2 * DESC_ROWS, D), ml_dtypes.bfloat16)
    xbf[:N_NODES] = x.astype(ml_dtypes.bfloat16)
    iota = np.tile(
        np.arange(P, dtype=np.float32).astype(ml_dtypes.bfloat16), (P, 1)
    )
    in_maps = []
    for k in range(N_CORES):
        m = ecore == k
        idxT, dstT = prep_core(
            src[m], erel[m], spans, caps, n_groups, TPG, NBUF_M
        )
        in_maps.append({"xbf": xbf, "idxT": idxT, "dstT": dstT, "iota": iota})

    trace = bool(int(os.environ.get("KERNEL_TRACE", "0")))
    res = run_bass_kernel_spmd(
        nc, in_maps, core_ids=list(range(N_CORES)), trace=trace
    )
    if trace:
        kernel.last_results = res
    dev = np.stack([res.results[c]["out"] for c in range(N_CORES)])  # [8, T*128, D]
    full = np.empty((N_NODES, D), np.float32)
    full[:] = dev.reshape(N_CORES * TILES * P, D)[node_slot]
    return np.ascontiguousarray(full)
